# revision 7
# baseline (speedup 1.0000x reference)
"""Conv1d (B=64, C_in=300, L=2048 -> C_out=512, K=3, pad=1) on 8 trn2 cores.

Strategy: data-parallel over batch (8 batches per core). Per batch, the conv
is computed as 9 accumulating matmuls per (co_chunk, l_chunk) PSUM tile:
3 ci-chunks (128/128/44) x 3 taps, each tap reading a shifted window of x
held in SBUF. x is zero-padded to length 2050 on the host so every tap is
a full 512-wide window at src offset k (FP32r matmuls require even column
counts and 8B-aligned PSUM offsets, so SBUF-side boundary trims are not
legal). Matmuls run in float32r (TF32 mode, full-rate). Bias is folded in
during PSUM evacuation on the vector engine.
"""

import contextlib

import numpy as np

import concourse.bass as bass
import concourse.mybir as mybir
import concourse.tile as tile
from concourse import bacc
from concourse.bass_utils import run_bass_kernel_spmd

B, C_IN, L = 64, 300, 2048
C_OUT, K = 512, 3
N_CORES = 8
B_LOC = B // N_CORES
CI_CHUNKS = [(0, 128), (128, 128), (256, 44)]
LP = L + 2  # host-side zero-padded length
N_COC = C_OUT // 128  # co chunks of 128 partitions
LC = 512  # l chunk = one PSUM bank of fp32
N_LC = L // LC
_NC_CACHE = {}


def _build_nc(reps=1):
    f32 = mybir.dt.float32
    f32r = mybir.dt.float32r
    nc = bacc.Bacc(None, target_bir_lowering=False)

    x_d = nc.dram_tensor("x", [B_LOC, C_IN, LP], f32r, kind="ExternalInput")
    w_d = nc.dram_tensor("w", [K, C_IN, C_OUT], f32r, kind="ExternalInput")
    b_d = nc.dram_tensor("b", [N_COC, 128, 1], f32, kind="ExternalInput")
    o_d = nc.dram_tensor("out", [B_LOC, C_OUT, L], f32, kind="ExternalOutput")

    with tile.TileContext(nc) as tc:
        with (
            tc.tile_pool(name="wpool", bufs=1) as wpool,
            tc.tile_pool(name="xpool", bufs=2) as xpool,
            tc.tile_pool(name="opool", bufs=3) as opool,
            tc.tile_pool(name="pspool", bufs=8, space="PSUM") as pspool,
            contextlib.ExitStack() as stack,
        ):
            w_sb = wpool.tile([128, K, len(CI_CHUNKS), C_OUT], f32r)
            for k in range(K):
                for cic, (c0, cs) in enumerate(CI_CHUNKS):
                    nc.sync.dma_start(
                        out=w_sb[0:cs, k, cic, :], in_=w_d[k, c0 : c0 + cs, :]
                    )
            bias_sb = wpool.tile([128, N_COC], f32)
            for coc in range(N_COC):
                nc.sync.dma_start(out=bias_sb[:, coc : coc + 1], in_=b_d[coc])

            if reps > 1:
                # Benchmark mode: repeat the whole body inside the NEFF so
                # per-iteration HW time can be isolated from RPC/transfer
                # overhead by differencing two rep counts.
                stack.enter_context(
                    tc.For_i(
                        0,
                        reps,
                        1,
                        hint_engines=(
                            mybir.EngineType.PE,
                            mybir.EngineType.DVE,
                            mybir.EngineType.SP,
                        ),
                    )
                )

            for b in range(B_LOC):
                x_sb = xpool.tile(
                    [128, len(CI_CHUNKS), LP], f32r, name="x_sb", tag="x"
                )
                for cic, (c0, cs) in enumerate(CI_CHUNKS):
                    nc.sync.dma_start(
                        out=x_sb[0:cs, cic, :], in_=x_d[b, c0 : c0 + cs, :]
                    )
                for coc in range(N_COC):
                    psums = [
                        pspool.tile([128, LC], f32, name="ps", tag="ps")
                        for _ in range(N_LC)
                    ]
                    n_acc = K * len(CI_CHUNKS)
                    acc = 0
                    # Weight-stationary: all 4 l-chunks per (k, ci-chunk).
                    for k in range(K):
                        for cic, (c0, cs) in enumerate(CI_CHUNKS):
                            lhsT = w_sb[
                                0:cs, k, cic, coc * 128 : (coc + 1) * 128
                            ]
                            for lc in range(N_LC):
                                # psum col j holds out[l0+j]; tap k reads
                                # x_pad[l0+j+k].
                                nc.tensor.matmul(
                                    psums[lc][:],
                                    lhsT,
                                    x_sb[0:cs, cic, lc * LC + k : lc * LC + k + LC],
                                    start=(acc == 0),
                                    stop=(acc == n_acc - 1),
                                )
                            acc += 1
                    out_sb = opool.tile([128, L], f32, name="out_sb", tag="o")
                    for lc in range(N_LC):
                        nc.vector.tensor_scalar_add(
                            out_sb[:, lc * LC : (lc + 1) * LC],
                            psums[lc][:],
                            bias_sb[:, coc : coc + 1],
                        )
                    nc.sync.dma_start(
                        out=o_d[b, coc * 128 : (coc + 1) * 128, :], in_=out_sb[:]
                    )

    nc.finalize()
    return nc


def _get_nc(reps=1):
    key = ("nc", reps)
    if key not in _NC_CACHE:
        _NC_CACHE[key] = _build_nc(reps)
    return _NC_CACHE[key]


def _run(inputs, trace=False, reps=1, **trace_kwargs):
    x = np.asarray(inputs["x"], dtype=np.float32)
    weight = np.asarray(inputs["weight"], dtype=np.float32)
    reg = np.asarray(inputs["words_regularization"], dtype=np.float32)
    bias = np.asarray(inputs["bias"], dtype=np.float32)

    w_eff = weight * reg[:, None, :]  # [C_out, C_in, K]
    wT = np.ascontiguousarray(w_eff.transpose(2, 1, 0))  # [K, C_in, C_out]
    b_r = np.ascontiguousarray(bias.reshape(N_COC, 128, 1))
    xp = np.pad(x, ((0, 0), (0, 0), (1, 1)))  # [B, C_in, LP]
    xs = xp.reshape(N_CORES, B_LOC, C_IN, LP)

    in_maps = [
        {"x": np.ascontiguousarray(xs[i]), "w": wT, "b": b_r}
        for i in range(N_CORES)
    ]
    nc = _get_nc(reps)
    res = run_bass_kernel_spmd(
        nc, in_maps, list(range(N_CORES)), trace=trace, **trace_kwargs
    )
    out = np.concatenate(
        [res.results[i]["out"] for i in range(N_CORES)], axis=0
    )
    return out, res


def kernel(**inputs):
    out, _ = _run(inputs, trace=False)
    return out


# revision 10
# speedup vs baseline: 1.0198x; 1.0198x over previous
"""Conv1d (B=64, C_in=300, L=2048 -> C_out=512, K=3, pad=1) on 8 trn2 cores.

Strategy: data-parallel over batch (8 batches per core). Per batch, the conv
is computed as 9 accumulating matmuls per (co_chunk, l_chunk) PSUM tile:
3 ci-chunks (128/128/44) x 3 taps, each tap reading a shifted window of x
held in SBUF. x is zero-padded to length 2050 on the host so every tap is
a full 512-wide window at src offset k (FP32r matmuls require even column
counts and 8B-aligned PSUM offsets, so SBUF-side boundary trims are not
legal). Matmuls run in float32r (TF32 mode, full-rate). Bias is folded in
during PSUM evacuation on the vector engine.
"""

import contextlib

import numpy as np

import concourse.bass as bass
import concourse.mybir as mybir
import concourse.tile as tile
from concourse import bacc
from concourse.bass_utils import run_bass_kernel_spmd

B, C_IN, L = 64, 300, 2048
C_OUT, K = 512, 3
N_CORES = 8
B_LOC = B // N_CORES
CI_CHUNKS = [(0, 128), (128, 128), (256, 44)]
LP = L + 2  # host-side zero-padded length
N_COC = C_OUT // 128  # co chunks of 128 partitions
LC = 512  # l chunk = one PSUM bank of fp32
N_LC = L // LC
_NC_CACHE = {}


def _build_nc(reps=1, probe=()):
    f32 = mybir.dt.float32
    f32r = mybir.dt.float32r
    nc = bacc.Bacc(None, target_bir_lowering=False)

    x_d = nc.dram_tensor("x", [B_LOC, C_IN, LP], f32r, kind="ExternalInput")
    w_d = nc.dram_tensor("w", [K, C_IN, C_OUT], f32r, kind="ExternalInput")
    b_d = nc.dram_tensor("b", [N_COC, 128, 1], f32, kind="ExternalInput")
    o_d = nc.dram_tensor("out", [B_LOC, C_OUT, L], f32, kind="ExternalOutput")

    with tile.TileContext(nc) as tc:
        with (
            tc.tile_pool(name="wpool", bufs=1) as wpool,
            tc.tile_pool(name="xpool", bufs=2) as xpool,
            tc.tile_pool(name="opool", bufs=3) as opool,
            tc.tile_pool(name="pspool", bufs=8, space="PSUM") as pspool,
            contextlib.ExitStack() as stack,
        ):
            w_sb = wpool.tile([128, K, len(CI_CHUNKS), C_OUT], f32r)
            for k in range(K):
                for cic, (c0, cs) in enumerate(CI_CHUNKS):
                    nc.sync.dma_start(
                        out=w_sb[0:cs, k, cic, :], in_=w_d[k, c0 : c0 + cs, :]
                    )
            bias_sb = wpool.tile([128, N_COC], f32)
            for coc in range(N_COC):
                nc.sync.dma_start(out=bias_sb[:, coc : coc + 1], in_=b_d[coc])

            if reps > 1:
                # Benchmark mode: repeat the whole body inside the NEFF so
                # per-iteration HW time can be isolated from RPC/transfer
                # overhead by differencing two rep counts.
                stack.enter_context(
                    tc.For_i(
                        0,
                        reps,
                        1,
                        hint_engines=(
                            mybir.EngineType.PE,
                            mybir.EngineType.DVE,
                            mybir.EngineType.SP,
                        ),
                    )
                )

            for b in range(B_LOC):
                x_sb = xpool.tile(
                    [128, len(CI_CHUNKS), LP], f32r, name="x_sb", tag="x"
                )
                # x1_sb holds unpadded x: the k=1 tap reads it at even
                # offset l0 (f32r moving operands stream slower from odd
                # 4B offsets).
                x1_sb = xpool.tile(
                    [128, len(CI_CHUNKS), L], f32r, name="x1_sb", tag="x1"
                )
                if "nodma" not in probe:
                    for cic, (c0, cs) in enumerate(CI_CHUNKS):
                        nc.sync.dma_start(
                            out=x_sb[0:cs, cic, :], in_=x_d[b, c0 : c0 + cs, :]
                        )
                        nc.sync.dma_start(
                            out=x1_sb[0:cs, cic, :],
                            in_=x_d[b, c0 : c0 + cs, 1 : L + 1],
                        )
                for coc in range(N_COC):
                    psums = [
                        pspool.tile([128, LC], f32, name="ps", tag="ps")
                        for _ in range(N_LC)
                    ]
                    n_acc = K * len(CI_CHUNKS)
                    acc = 0
                    # Weight-stationary: all 4 l-chunks per (k, ci-chunk).
                    for k in range(K):
                        for cic, (c0, cs) in enumerate(CI_CHUNKS):
                            kw, cicw = (0, 0) if "samew" in probe else (k, cic)
                            lhsT = w_sb[
                                0:cs, kw, cicw, coc * 128 : (coc + 1) * 128
                            ]
                            for lc in range(N_LC):
                                # psum col j holds out[l0+j]; tap k reads
                                # x_pad[l0+j+k]; for k=1 that is x[l0+j],
                                # read from the unpadded copy at even
                                # offset.
                                if k == 1 and "off0" not in probe:
                                    rhs = x1_sb[
                                        0:cs, cic, lc * LC : lc * LC + LC
                                    ]
                                else:
                                    koff = 0 if "off0" in probe else k
                                    rhs = x_sb[
                                        0:cs,
                                        cic,
                                        lc * LC + koff : lc * LC + koff + LC,
                                    ]
                                nc.tensor.matmul(
                                    psums[lc][:],
                                    lhsT,
                                    rhs,
                                    start=(acc == 0),
                                    stop=(acc == n_acc - 1),
                                )
                            acc += 1
                    out_sb = opool.tile([128, L], f32, name="out_sb", tag="o")
                    for lc in range(N_LC):
                        nc.vector.tensor_scalar_add(
                            out_sb[:, lc * LC : (lc + 1) * LC],
                            psums[lc][:],
                            bias_sb[:, coc : coc + 1],
                        )
                    nc.sync.dma_start(
                        out=o_d[b, coc * 128 : (coc + 1) * 128, :], in_=out_sb[:]
                    )

    nc.finalize()
    return nc


def _get_nc(reps=1, probe=()):
    key = ("nc", reps, tuple(probe))
    if key not in _NC_CACHE:
        _NC_CACHE[key] = _build_nc(reps, probe)
    return _NC_CACHE[key]


def _run(inputs, trace=False, reps=1, **trace_kwargs):
    x = np.asarray(inputs["x"], dtype=np.float32)
    weight = np.asarray(inputs["weight"], dtype=np.float32)
    reg = np.asarray(inputs["words_regularization"], dtype=np.float32)
    bias = np.asarray(inputs["bias"], dtype=np.float32)

    w_eff = weight * reg[:, None, :]  # [C_out, C_in, K]
    wT = np.ascontiguousarray(w_eff.transpose(2, 1, 0))  # [K, C_in, C_out]
    b_r = np.ascontiguousarray(bias.reshape(N_COC, 128, 1))
    xp = np.pad(x, ((0, 0), (0, 0), (1, 1)))  # [B, C_in, LP]
    xs = xp.reshape(N_CORES, B_LOC, C_IN, LP)

    in_maps = [
        {"x": np.ascontiguousarray(xs[i]), "w": wT, "b": b_r}
        for i in range(N_CORES)
    ]
    nc = _get_nc(reps)
    res = run_bass_kernel_spmd(
        nc, in_maps, list(range(N_CORES)), trace=trace, **trace_kwargs
    )
    out = np.concatenate(
        [res.results[i]["out"] for i in range(N_CORES)], axis=0
    )
    return out, res


def kernel(**inputs):
    out, _ = _run(inputs, trace=False)
    return out
